# revision 16
# baseline (speedup 1.0000x reference)
"""Banded (sparse) attention + projections on 8 Trainium2 NeuronCores.

Problem: nn_Attention_old_90211493085279
  x [2, 2048, 1024] -> qkv = x @ Wqkv, banded softmax(QK^T) V (half-width 8),
  out = attn @ Wproj + bproj.

Sharding choice: shard (batch x tokens) across the 8 cores -- each core owns a
contiguous block of 512 token rows (2 batches x 4 quarters). Because the
attention band is only 17 wide, each core needs just an 8-token halo of K/V
context, so there are NO collectives: every core computes QKV for its halo'd
token range (528 tokens), all 16 heads of banded attention for its own 512
rows, and the full output projection for its rows. The host concatenates the
per-core [1024, 512] transposed outputs.

Internal layouts (per core):
  qkT  [2048, 528]  q (pre-scaled by 1/8, folded into Wq) and k, [feature, token]
  v1   [528, 16*65] v in natural [token, (head, dim+1)] layout; 65th column = 1.0
                    so the AV matmul also produces the softmax denominator.
  scores are computed transposed: st[w, p] = sum_d k[d, w] q[d, p]; softmax
  weights at = exp(st) * bandmask (0/1 masks, per-core data => SPMD-uniform).
  O^T [65, 128] = v1^T @ at  (row 64 = denominators); normalization via
  reciprocal + PE rank-1 broadcast; projection consumes O^T directly.
"""

import sys

sys.path.insert(0, "/opt/trn_rl_repo")

import numpy as np

import concourse.bass as bass
import concourse.tile as tile
from concourse import bacc, mybir
from concourse.bass_utils import run_bass_kernel_spmd

F32 = mybir.dt.float32
AF = mybir.ActivationFunctionType

B, N, C, H, HD, W = 2, 2048, 1024, 16, 64, 8
SCALE = float(HD) ** -0.5
CORES = 8
TOK = 512            # token rows owned per core
HALO = TOK + 2 * W   # 528 k/v context tokens per core
NT = TOK // 128      # 4 row tiles of 128
WIN = 128 + 2 * W    # 144 k/v window per row tile

_CACHE = {}


def _build_nc(dbg=False):
    nc = bacc.Bacc(None, target_bir_lowering=False)
    xhT = nc.dram_tensor("xhT", [C, HALO], F32, kind="ExternalInput")
    wqk = nc.dram_tensor("wqk", [C, 2 * C], F32, kind="ExternalInput")
    wv = nc.dram_tensor("wv", [C, C], F32, kind="ExternalInput")
    wp = nc.dram_tensor("wp", [C, C], F32, kind="ExternalInput")
    bp = nc.dram_tensor("bp", [128, 8], F32, kind="ExternalInput")
    m1 = nc.dram_tensor("m1", [NT, 128, 128], F32, kind="ExternalInput")
    m2 = nc.dram_tensor("m2", [NT, 2 * W, 128], F32, kind="ExternalInput")
    outT = nc.dram_tensor("outT", [C, TOK], F32, kind="ExternalOutput")
    if dbg:
        qkTo = nc.dram_tensor("qkTo", [2 * C, HALO], F32, kind="ExternalOutput")
        v1o = nc.dram_tensor("v1o", [HALO, H * (HD + 1)], F32, kind="ExternalOutput")
        otno = nc.dram_tensor("otno", [C, TOK], F32, kind="ExternalOutput")
        s_allo = nc.dram_tensor("s_allo", [64, 128], F32, kind="ExternalOutput")

    vsizes = [128, 128, 128, 128, 2 * W]

    with tile.TileContext(nc) as tc:
        with tc.tile_pool(name="persist", bufs=1) as pp:
            # ---- persistent SBUF arrays ----
            xh = [pp.tile([128, HALO], F32, tag=f"xh{c}", name=f"xh{c}") for c in range(8)]
            for c in range(8):
                nc.sync.dma_start(out=xh[c][:], in_=xhT[128 * c:128 * (c + 1), :])
            wv_sb = [pp.tile([128, C], F32, tag=f"wv{c}", name=f"wv{c}") for c in range(8)]
            for c in range(8):
                nc.sync.dma_start(out=wv_sb[c][:], in_=wv[128 * c:128 * (c + 1), :])
            mask1 = [pp.tile([128, 128], F32, tag=f"mk1_{t}", name=f"mk1_{t}") for t in range(NT)]
            mask2 = [pp.tile([2 * W, 128], F32, tag=f"mk2_{t}", name=f"mk2_{t}") for t in range(NT)]
            for t in range(NT):
                nc.sync.dma_start(out=mask1[t][:], in_=m1[t])
                nc.sync.dma_start(out=mask2[t][:], in_=m2[t])
            bias_sb = pp.tile([128, 8], F32, tag="bias", name="bias")
            nc.sync.dma_start(out=bias_sb[:], in_=bp[:])

            qkT = [pp.tile([128, HALO], F32, tag=f"qkT{m}", name=f"qkT{m}") for m in range(16)]
            v1 = [pp.tile([p, H, HD + 1], F32, tag=f"v1_{t}", name=f"v1_{t}")
                  for t, p in enumerate(vsizes)]
            otn = [pp.tile([128, TOK], F32, tag=f"otn{m}", name=f"otn{m}") for m in range(8)]
            s_all = pp.tile([64, 128], F32, tag="s_all", name="s_all")
            rec = pp.tile([64, 128], F32, tag="rec", name="rec")

            # ---- P1: q,k projection -> qkT[m] = (Wqk[:, m-chunk]).T @ x^T ----
            with tc.tile_pool(name="wqkp", bufs=6) as wqkp, \
                 tc.tile_pool(name="psA", bufs=2, space="PSUM") as psA, \
                 tc.tile_pool(name="psB", bufs=2, space="PSUM") as psB:
                for m in range(16):
                    pa = psA.tile([128, 264], F32, tag="pa", name="pa")
                    pb = psB.tile([128, 264], F32, tag="pb", name="pb")
                    for c in range(8):
                        wt = wqkp.tile([128, 128], F32, tag="wt", name="wt")
                        nc.sync.dma_start(
                            out=wt[:],
                            in_=wqk[128 * c:128 * (c + 1), 128 * m:128 * (m + 1)])
                        nc.tensor.matmul(pa[:], wt[:], xh[c][:, 0:264],
                                         start=(c == 0), stop=(c == 7))
                        nc.tensor.matmul(pb[:], wt[:], xh[c][:, 264:528],
                                         start=(c == 0), stop=(c == 7))
                    nc.vector.tensor_copy(qkT[m][:, 0:264], pa[:])
                    nc.vector.tensor_copy(qkT[m][:, 264:528], pb[:])

            # ---- P2: v projection (natural layout) + ones column ----
            with tc.tile_pool(name="psV", bufs=4, space="PSUM") as psV:
                for t in range(5):
                    p = vsizes[t]
                    pv0 = psV.tile([128, 512], F32, tag="pv0", name="pv0")
                    pv1 = psV.tile([128, 512], F32, tag="pv1", name="pv1")
                    for c in range(8):
                        lhs = xh[c][:, 128 * t:128 * t + p]
                        nc.tensor.matmul(pv0[:p, :], lhs, wv_sb[c][:, 0:512],
                                         start=(c == 0), stop=(c == 7))
                        nc.tensor.matmul(pv1[:p, :], lhs, wv_sb[c][:, 512:1024],
                                         start=(c == 0), stop=(c == 7))
                    nc.vector.tensor_copy(
                        v1[t][:, 0:8, 0:HD],
                        pv0[:p, :].rearrange("p (h d) -> p h d", d=HD))
                    nc.vector.tensor_copy(
                        v1[t][:, 8:16, 0:HD],
                        pv1[:p, :].rearrange("p (h d) -> p h d", d=HD))
                    nc.vector.memset(v1[t][:, :, HD], 1.0)

            # ---- P3: banded attention, scores transposed ----
            with tc.tile_pool(name="ps1", bufs=2, space="PSUM") as ps1, \
                 tc.tile_pool(name="ps2", bufs=2, space="PSUM") as ps2, \
                 tc.tile_pool(name="pso", bufs=2, space="PSUM") as pso, \
                 tc.tile_pool(name="atp", bufs=3) as atp, \
                 tc.tile_pool(name="atp2", bufs=3) as atp2, \
                 tc.tile_pool(name="stp", bufs=4) as stp:
                for h in range(H):
                    fm = h // 2
                    off = (h % 2) * 64
                    for t in range(NT):
                        st1 = ps1.tile([128, 128], F32, tag="st1", name="st1")
                        st2 = ps2.tile([2 * W, 128], F32, tag="st2", name="st2")
                        q_ap = qkT[fm][off:off + 64, W + 128 * t:W + 128 * t + 128]
                        k1 = qkT[8 + fm][off:off + 64, 128 * t:128 * t + 128]
                        k2 = qkT[8 + fm][off:off + 64, 128 * t + 128:128 * t + WIN]
                        nc.tensor.matmul(st1[:], k1, q_ap, start=True, stop=True)
                        nc.tensor.matmul(st2[:], k2, q_ap, start=True, stop=True)
                        at1 = atp.tile([128, 128], F32, tag="at1", name="at1")
                        at2 = atp2.tile([2 * W, 128], F32, tag="at2", name="at2")
                        nc.scalar.activation(at1[:], st1[:], AF.Exp)
                        nc.scalar.activation(at2[:], st2[:], AF.Exp)
                        nc.vector.tensor_mul(at1[:], at1[:], mask1[t][:])
                        nc.vector.tensor_mul(at2[:], at2[:], mask2[t][:])
                        ot = pso.tile([HD + 1, 128], F32, tag="ot", name="ot")
                        nc.tensor.matmul(ot[:], v1[t][:, h, :], at1[:],
                                         start=True, stop=False)
                        nc.tensor.matmul(ot[:], v1[t + 1][0:2 * W, h, :], at2[:],
                                         start=False, stop=True)
                        stage = stp.tile([HD + 1, 128], F32, tag="stage", name="stage")
                        nc.scalar.copy(stage[:], ot[:])
                        nc.sync.dma_start(
                            out=otn[fm][off:off + 64, 128 * t:128 * (t + 1)],
                            in_=stage[0:HD, :])
                        nc.sync.dma_start(out=s_all[4 * h + t:4 * h + t + 1, :],
                                          in_=stage[HD:HD + 1, :])

            # ---- P4: normalization: otn *= broadcast(1/s_all) ----
            nc.vector.reciprocal(rec[:], s_all[:])
            dp = tc.alloc_tile_pool(name="dram", bufs=1, space="DRAM")
            drec = dp.tile([64, 128], F32, tag="drec", name="drec")
            nc.sync.dma_start(out=drec[:], in_=rec[:])
            with tc.tile_pool(name="bcp", bufs=4) as bcp:
                for fm in range(8):
                    for t in range(NT):
                        bc = bcp.tile([128, 128], F32, tag="bc", name="bc")
                        nc.sync.dma_start(
                            out=bc[0:64, :],
                            in_=drec[4 * (2 * fm) + t:4 * (2 * fm) + t + 1, :]
                            .to_broadcast((64, 128)))
                        nc.sync.dma_start(
                            out=bc[64:128, :],
                            in_=drec[4 * (2 * fm + 1) + t:4 * (2 * fm + 1) + t + 1, :]
                            .to_broadcast((64, 128)))
                        sl = otn[fm][:, 128 * t:128 * (t + 1)]
                        nc.vector.tensor_mul(sl, sl, bc[:])

            # ---- P5: output projection (transposed) + bias ----
            with tc.tile_pool(name="wpp", bufs=6) as wpp, \
                 tc.tile_pool(name="psf", bufs=2, space="PSUM") as psf, \
                 tc.tile_pool(name="outp", bufs=2) as outp:
                for m in range(8):
                    pf = psf.tile([128, 512], F32, tag="pf", name="pf")
                    for c in range(8):
                        wt2 = wpp.tile([128, 128], F32, tag="wt2", name="wt2")
                        nc.sync.dma_start(
                            out=wt2[:],
                            in_=wp[128 * c:128 * (c + 1), 128 * m:128 * (m + 1)])
                        nc.tensor.matmul(pf[:], wt2[:], otn[c][:],
                                         start=(c == 0), stop=(c == 7))
                    ob = outp.tile([128, 512], F32, tag="ob", name="ob")
                    nc.vector.tensor_scalar_add(ob[:], pf[:], bias_sb[:, m:m + 1])
                    nc.sync.dma_start(out=outT[128 * m:128 * (m + 1), :], in_=ob[:])

            if dbg:
                for m in range(16):
                    nc.sync.dma_start(out=qkTo[128 * m:128 * (m + 1), :], in_=qkT[m][:])
                for t in range(5):
                    p = vsizes[t]
                    nc.sync.dma_start(
                        out=v1o[128 * t:128 * t + p, :],
                        in_=v1[t][:].rearrange("p h d -> p (h d)"))
                for m in range(8):
                    nc.sync.dma_start(out=otno[128 * m:128 * (m + 1), :], in_=otn[m][:])
                nc.sync.dma_start(out=s_allo[:], in_=s_all[:])

    nc.finalize()
    return nc


def _get_nc(dbg=False):
    key = ("nc", dbg)
    if key not in _CACHE:
        _CACHE[key] = _build_nc(dbg)
    return _CACHE[key]


def _band_mask_np(n, w):
    i = np.arange(n)[:, None]
    j = np.arange(n)[None, :]
    lo = np.where(i <= w, 0, i - w)
    hi = np.where(n - i <= w, n - 1, i + w)
    return (j >= lo) & (j <= hi)


def _make_in_maps(x, Wqkv, Wproj, bproj):
    x = np.ascontiguousarray(np.asarray(x, dtype=np.float32))
    Wqkv = np.asarray(Wqkv, dtype=np.float32)
    Wproj = np.ascontiguousarray(np.asarray(Wproj, dtype=np.float32))
    bproj = np.asarray(bproj, dtype=np.float32)

    wqk_host = np.concatenate(
        [Wqkv[:, :C] * np.float32(SCALE), Wqkv[:, C:2 * C]], axis=1)
    wqk_host = np.ascontiguousarray(wqk_host)
    wv_host = np.ascontiguousarray(Wqkv[:, 2 * C:])
    bp_host = np.ascontiguousarray(bproj.reshape(8, 128).T)
    band = _band_mask_np(N, W)

    in_maps = []
    for core in range(CORES):
        b, qt = divmod(core, NT)
        g0 = qt * TOK
        xhrows = np.zeros((HALO, C), np.float32)
        s = max(0, g0 - W)
        e = min(N, g0 + TOK + W)
        xhrows[s - (g0 - W):e - (g0 - W)] = x[b, s:e]
        xhT_host = np.ascontiguousarray(xhrows.T)

        m1 = np.zeros((NT, 128, 128), np.float32)
        m2 = np.zeros((NT, 2 * W, 128), np.float32)
        for t in range(NT):
            i = g0 + 128 * t + np.arange(128)[None, :]
            jw = (g0 - W) + 128 * t + np.arange(WIN)[:, None]
            valid = (jw >= 0) & (jw < N)
            mm = band[i, np.clip(jw, 0, N - 1)] & valid
            m1[t] = mm[:128]
            m2[t] = mm[128:]
        in_maps.append({
            "xhT": xhT_host, "wqk": wqk_host, "wv": wv_host,
            "wp": Wproj, "bp": bp_host, "m1": m1, "m2": m2,
        })
    return in_maps


def run_spmd(x, Wqkv, Wproj, bproj, dbg=False, **kw):
    """Run the SPMD kernel; returns (output, BassKernelResults)."""
    nc = _get_nc(dbg)
    in_maps = _make_in_maps(x, Wqkv, Wproj, bproj)
    res = run_bass_kernel_spmd(nc, in_maps, list(range(CORES)), **kw)
    outT = np.concatenate([res.results[i]["outT"] for i in range(CORES)], axis=1)
    out = np.ascontiguousarray(outT.T).reshape(B, N, C)
    return out, res


def kernel(x, Wqkv, Wproj, bproj):
    out, _ = run_spmd(x, Wqkv, Wproj, bproj)
    return out


# revision 19
# speedup vs baseline: 1.6573x; 1.6573x over previous
"""Banded (sparse) attention + projections on 8 Trainium2 NeuronCores.

Problem: nn_Attention_old_90211493085279
  x [2, 2048, 1024] -> qkv = x @ Wqkv, banded softmax(QK^T) V (half-width 8),
  out = attn @ Wproj + bproj.

Sharding choice: shard (batch x tokens) across the 8 cores -- each core owns a
contiguous block of 512 token rows (2 batches x 4 quarters). Because the
attention band is only 17 wide, each core needs just an 8-token halo of K/V
context, so there are NO collectives: every core computes QKV for its halo'd
token range (528 tokens), all 16 heads of banded attention for its own 512
rows, and the full output projection for its rows. The host concatenates the
per-core [1024, 512] transposed outputs.

Internal layouts (per core):
  qkT  [2048, 528]  q (pre-scaled by 1/8, folded into Wq) and k, [feature, token]
  v1   [528, 16*65] v in natural [token, (head, dim+1)] layout; 65th column = 1.0
                    so the AV matmul also produces the softmax denominator.
  scores are computed transposed: st[w, p] = sum_d k[d, w] q[d, p]; softmax
  weights at = exp(st) * bandmask (0/1 masks, per-core data => SPMD-uniform).
  O^T [65, 128] = v1^T @ at  (row 64 = denominators); normalization via
  reciprocal + PE rank-1 broadcast; projection consumes O^T directly.
"""

import sys

sys.path.insert(0, "/opt/trn_rl_repo")

import ml_dtypes
import numpy as np

import concourse.bass as bass
import concourse.tile as tile
from concourse import bacc, mybir
from concourse.bass_utils import run_bass_kernel_spmd

F32 = mybir.dt.float32
BF16 = mybir.dt.bfloat16
AF = mybir.ActivationFunctionType

B, N, C, H, HD, W = 2, 2048, 1024, 16, 64, 8
SCALE = float(HD) ** -0.5
CORES = 8
TOK = 512            # token rows owned per core
HALO = TOK + 2 * W   # 528 k/v context tokens per core
NT = TOK // 128      # 4 row tiles of 128
WIN = 128 + 2 * W    # 144 k/v window per row tile

_CACHE = {}


def _build_nc(dbg=False):
    nc = bacc.Bacc(None, target_bir_lowering=False)
    xhT = nc.dram_tensor("xhT", [C, HALO], BF16, kind="ExternalInput")
    wqk = nc.dram_tensor("wqk", [C, 2 * C], BF16, kind="ExternalInput")
    wv = nc.dram_tensor("wv", [C, C], BF16, kind="ExternalInput")
    wp = nc.dram_tensor("wp", [C, C], BF16, kind="ExternalInput")
    bp = nc.dram_tensor("bp", [128, 8], F32, kind="ExternalInput")
    mC = nc.dram_tensor("mC", [NT, 128, 2 * 128], BF16, kind="ExternalInput")
    outT = nc.dram_tensor("outT", [C, TOK], F32, kind="ExternalOutput")
    if dbg:
        qkTo = nc.dram_tensor("qkTo", [2 * C, HALO], BF16, kind="ExternalOutput")
        v1o = nc.dram_tensor("v1o", [HALO, H * (HD + 1)], BF16, kind="ExternalOutput")
        otno = nc.dram_tensor("otno", [C, TOK], BF16, kind="ExternalOutput")
        s_allo = nc.dram_tensor("s_allo", [64, 128], F32, kind="ExternalOutput")

    vsizes = [128, 128, 128, 128, 2 * W]

    def R(ap):
        return ap

    with tile.TileContext(nc) as tc:
        with tc.tile_pool(name="persist", bufs=1) as pp:
            # ---- persistent SBUF arrays ----
            xh = [pp.tile([128, HALO], BF16, tag=f"xh{c}", name=f"xh{c}") for c in range(8)]
            for c in range(8):
                nc.sync.dma_start(out=xh[c][:], in_=xhT[128 * c:128 * (c + 1), :])
            wv_sb = [pp.tile([128, C], BF16, tag=f"wv{c}", name=f"wv{c}") for c in range(8)]
            for c in range(8):
                nc.sync.dma_start(out=wv_sb[c][:], in_=wv[128 * c:128 * (c + 1), :])
            maskC = [pp.tile([128, 2 * 128], BF16, tag=f"mkC_{t}", name=f"mkC_{t}") for t in range(NT)]
            for t in range(NT):
                nc.sync.dma_start(out=maskC[t][:], in_=mC[t])
            bias_sb = pp.tile([128, 8], F32, tag="bias", name="bias")
            nc.sync.dma_start(out=bias_sb[:], in_=bp[:])

            qkT = [pp.tile([128, HALO], BF16, tag=f"qkT{m}", name=f"qkT{m}") for m in range(16)]
            v1 = [pp.tile([p, H, HD + 1], BF16, tag=f"v1_{t}", name=f"v1_{t}")
                  for t, p in enumerate(vsizes)]
            otn = [pp.tile([128, TOK], BF16, tag=f"otn{m}", name=f"otn{m}") for m in range(8)]
            s_all = pp.tile([64, 128], F32, tag="s_all", name="s_all")
            rec = pp.tile([64, 128], F32, tag="rec", name="rec")

            # ---- P1: q,k projection -> qkT[m] = (Wqk[:, m-chunk]).T @ x^T ----
            with tc.tile_pool(name="wqkp", bufs=6) as wqkp, \
                 tc.tile_pool(name="psA", bufs=2, space="PSUM") as psA, \
                 tc.tile_pool(name="psB", bufs=2, space="PSUM") as psB:
                for m in range(16):
                    pa = psA.tile([128, 264], F32, tag="pa", name="pa")
                    pb = psB.tile([128, 264], F32, tag="pb", name="pb")
                    for c in range(8):
                        wt = wqkp.tile([128, 128], BF16, tag="wt", name="wt")
                        nc.sync.dma_start(
                            out=wt[:],
                            in_=wqk[128 * c:128 * (c + 1), 128 * m:128 * (m + 1)])
                        nc.tensor.matmul(pa[:], R(wt[:]), R(xh[c][:, 0:264]),
                                         start=(c == 0), stop=(c == 7))
                        nc.tensor.matmul(pb[:], R(wt[:]), R(xh[c][:, 264:528]),
                                         start=(c == 0), stop=(c == 7))
                    nc.vector.tensor_copy(qkT[m][:, 0:264], pa[:])
                    nc.vector.tensor_copy(qkT[m][:, 264:528], pb[:])

            # ---- P2: v projection (natural layout) + ones column ----
            with tc.tile_pool(name="psV", bufs=4, space="PSUM") as psV:
                for t in range(5):
                    p = vsizes[t]
                    pv0 = psV.tile([128, 512], F32, tag="pv0", name="pv0")
                    pv1 = psV.tile([128, 512], F32, tag="pv1", name="pv1")
                    for c in range(8):
                        lhs = xh[c][:, 128 * t:128 * t + p]
                        nc.tensor.matmul(pv0[:p, :], R(lhs), R(wv_sb[c][:, 0:512]),
                                         start=(c == 0), stop=(c == 7))
                        nc.tensor.matmul(pv1[:p, :], R(lhs), R(wv_sb[c][:, 512:1024]),
                                         start=(c == 0), stop=(c == 7))
                    nc.vector.tensor_copy(
                        v1[t][:, 0:8, 0:HD],
                        pv0[:p, :].rearrange("p (h d) -> p h d", d=HD))
                    nc.vector.tensor_copy(
                        v1[t][:, 8:16, 0:HD],
                        pv1[:p, :].rearrange("p (h d) -> p h d", d=HD))
                    nc.vector.memset(v1[t][:, :, HD], 1.0)

            # ---- P3: banded attention, scores transposed ----
            with tc.tile_pool(name="ps1", bufs=3, space="PSUM") as ps1, \
                 tc.tile_pool(name="pso", bufs=3, space="PSUM") as pso, \
                 tc.tile_pool(name="atp", bufs=3) as atp, \
                 tc.tile_pool(name="srp", bufs=3) as srp:
                for h in range(H):
                    fm = h // 2
                    off = (h % 2) * 64
                    for t in range(NT):
                        st = ps1.tile([128, 2 * 128], F32, tag="st", name="st")
                        q_ap = R(qkT[fm][off:off + 64, W + 128 * t:W + 128 * t + 128])
                        k1 = R(qkT[8 + fm][off:off + 64, 128 * t:128 * t + 128])
                        k2 = R(qkT[8 + fm][off:off + 64, 128 * t + 128:128 * t + WIN])
                        nc.tensor.matmul(st[:, 0:128], k1, q_ap, start=True, stop=True)
                        nc.tensor.matmul(st[0:2 * W, 128:256], k2, q_ap,
                                         start=True, stop=True)
                        at = atp.tile([128, 2 * 128], BF16, tag="at", name="at")
                        nc.scalar.activation(at[:], st[:], AF.Exp)
                        nc.vector.tensor_mul(at[:], at[:], maskC[t][:])
                        ot = pso.tile([HD + 1, 128], F32, tag="ot", name="ot")
                        nc.tensor.matmul(ot[:], R(v1[t][:, h, :]), R(at[:, 0:128]),
                                         start=True, stop=False)
                        nc.tensor.matmul(ot[:], R(v1[t + 1][0:2 * W, h, :]),
                                         R(at[0:2 * W, 128:256]),
                                         start=False, stop=True)
                        nc.vector.tensor_copy(
                            otn[fm][off:off + 64, 128 * t:128 * (t + 1)], ot[0:HD, :])
                        srow = srp.tile([HD + 1, 128], F32, tag="srow", name="srow")
                        nc.scalar.copy(srow[HD:HD + 1, :], ot[HD:HD + 1, :])
                        nc.sync.dma_start(out=s_all[4 * h + t:4 * h + t + 1, :],
                                          in_=srow[HD:HD + 1, :])

            # ---- P4: normalization: otn *= broadcast(1/s_all) ----
            nc.vector.reciprocal(rec[:], s_all[:])
            dp = tc.alloc_tile_pool(name="dram", bufs=1, space="DRAM")
            drec = dp.tile([64, 128], F32, tag="drec", name="drec")
            nc.sync.dma_start(out=drec[:], in_=rec[:])
            with tc.tile_pool(name="bcp", bufs=4) as bcp:
                dr_ap = drec[:]
                for fm in range(8):
                    bc = bcp.tile([128, TOK], F32, tag="bc", name="bc")
                    for j in range(2):
                        src = bass.AP(
                            tensor=dr_ap.tensor,
                            offset=dr_ap.offset + (8 * fm + 4 * j) * 128,
                            ap=[[0, 64], [128, NT], [1, 128]])
                        dst = bc[64 * j:64 * (j + 1), :].rearrange(
                            "b (t c) -> b t c", t=NT)
                        nc.sync.dma_start(out=dst, in_=src)
                    nc.vector.tensor_mul(otn[fm][:], otn[fm][:], bc[:])

            # ---- P5: output projection (transposed) + bias ----
            with tc.tile_pool(name="wpp", bufs=6) as wpp, \
                 tc.tile_pool(name="psf", bufs=2, space="PSUM") as psf, \
                 tc.tile_pool(name="outp", bufs=2) as outp:
                for m in range(8):
                    pf = psf.tile([128, 512], F32, tag="pf", name="pf")
                    for c in range(8):
                        wt2 = wpp.tile([128, 128], BF16, tag="wt2", name="wt2")
                        nc.sync.dma_start(
                            out=wt2[:],
                            in_=wp[128 * c:128 * (c + 1), 128 * m:128 * (m + 1)])
                        nc.tensor.matmul(pf[:], R(wt2[:]), R(otn[c][:]),
                                         start=(c == 0), stop=(c == 7))
                    ob = outp.tile([128, 512], F32, tag="ob", name="ob")
                    nc.vector.tensor_scalar_add(ob[:], pf[:], bias_sb[:, m:m + 1])
                    nc.sync.dma_start(out=outT[128 * m:128 * (m + 1), :], in_=ob[:])

            if dbg:
                for m in range(16):
                    nc.sync.dma_start(out=qkTo[128 * m:128 * (m + 1), :], in_=qkT[m][:])
                for t in range(5):
                    p = vsizes[t]
                    nc.sync.dma_start(
                        out=v1o[128 * t:128 * t + p, :],
                        in_=v1[t][:].rearrange("p h d -> p (h d)"))
                for m in range(8):
                    nc.sync.dma_start(out=otno[128 * m:128 * (m + 1), :], in_=otn[m][:])
                nc.sync.dma_start(out=s_allo[:], in_=s_all[:])

    nc.finalize()
    return nc


def _get_nc(dbg=False):
    key = ("nc", dbg)
    if key not in _CACHE:
        _CACHE[key] = _build_nc(dbg)
    return _CACHE[key]


def _band_mask_np(n, w):
    i = np.arange(n)[:, None]
    j = np.arange(n)[None, :]
    lo = np.where(i <= w, 0, i - w)
    hi = np.where(n - i <= w, n - 1, i + w)
    return (j >= lo) & (j <= hi)


def _make_in_maps(x, Wqkv, Wproj, bproj):
    x = np.ascontiguousarray(np.asarray(x, dtype=np.float32))
    Wqkv = np.asarray(Wqkv, dtype=np.float32)
    Wproj = np.ascontiguousarray(np.asarray(Wproj, dtype=np.float32))
    bproj = np.asarray(bproj, dtype=np.float32)

    wqk_host = np.concatenate(
        [Wqkv[:, :C] * np.float32(SCALE), Wqkv[:, C:2 * C]], axis=1)
    wqk_host = np.ascontiguousarray(wqk_host).astype(ml_dtypes.bfloat16)
    wv_host = np.ascontiguousarray(Wqkv[:, 2 * C:]).astype(ml_dtypes.bfloat16)
    bp_host = np.ascontiguousarray(bproj.reshape(8, 128).T)
    band = _band_mask_np(N, W)

    in_maps = []
    for core in range(CORES):
        b, qt = divmod(core, NT)
        g0 = qt * TOK
        xhrows = np.zeros((HALO, C), np.float32)
        s = max(0, g0 - W)
        e = min(N, g0 + TOK + W)
        xhrows[s - (g0 - W):e - (g0 - W)] = x[b, s:e]
        xhT_host = np.ascontiguousarray(xhrows.T).astype(ml_dtypes.bfloat16)

        mCh = np.zeros((NT, 128, 2 * 128), np.float32)
        for t in range(NT):
            i = g0 + 128 * t + np.arange(128)[None, :]
            jw = (g0 - W) + 128 * t + np.arange(WIN)[:, None]
            valid = (jw >= 0) & (jw < N)
            mm = band[i, np.clip(jw, 0, N - 1)] & valid
            mCh[t][:, 0:128] = mm[:128]
            mCh[t][0:2 * W, 128:256] = mm[128:]
        in_maps.append({
            "xhT": xhT_host, "wqk": wqk_host, "wv": wv_host,
            "wp": Wproj.astype(ml_dtypes.bfloat16), "bp": bp_host,
            "mC": mCh.astype(ml_dtypes.bfloat16),
        })
    return in_maps


def run_spmd(x, Wqkv, Wproj, bproj, dbg=False, **kw):
    """Run the SPMD kernel; returns (output, BassKernelResults)."""
    nc = _get_nc(dbg)
    in_maps = _make_in_maps(x, Wqkv, Wproj, bproj)
    res = run_bass_kernel_spmd(nc, in_maps, list(range(CORES)), **kw)
    outT = np.concatenate([res.results[i]["outT"] for i in range(CORES)], axis=1)
    out = np.ascontiguousarray(outT.T).reshape(B, N, C)
    return out, res


def kernel(x, Wqkv, Wproj, bproj):
    out, _ = run_spmd(x, Wqkv, Wproj, bproj)
    return out


# revision 20
# speedup vs baseline: 2.9489x; 1.7793x over previous
"""Banded (sparse) attention + projections on 8 Trainium2 NeuronCores.

Problem: nn_Attention_old_90211493085279
  x [2, 2048, 1024] -> qkv = x @ Wqkv, banded softmax(QK^T) V (half-width 8),
  out = attn @ Wproj + bproj.

Sharding choice: shard (batch x tokens) across the 8 cores -- each core owns a
contiguous block of 512 token rows (2 batches x 4 quarters). Because the
attention band is only 17 wide, each core needs just an 8-token halo of K/V
context, so there are NO collectives: every core computes QKV for its halo'd
token range (528 tokens), all 16 heads of banded attention for its own 512
rows, and the full output projection for its rows. The host concatenates the
per-core [1024, 512] transposed outputs.

Per-core layouts (matmul operands bf16, accumulation f32):
  qkT  [2048, 528]  q (pre-scaled by 1/8, folded into Wq) and k, [feature, token]
  v1   [528, 16*65] v in natural [token, (head, dim+1)] layout; 65th column = 1.0
                    so the AV matmul also produces the softmax denominator.
  scores per head computed transposed into [window, 4*128 rows] PSUM strips;
  at = exp(st) * bandmask (0/1 bf16 masks, per-core data => SPMD-uniform).
  O^T strip [65, 512] = v1^T @ at per head (row 64 = denominators);
  normalization: sums -> DRAM -> reciprocal -> DRAM-broadcast -> one mul/head
  pair; the projection consumes O^T directly; host re-transposes.
"""

import sys

sys.path.insert(0, "/opt/trn_rl_repo")

import ml_dtypes
import numpy as np

import concourse.bass as bass
import concourse.tile as tile
from concourse import bacc, mybir
from concourse.bass_utils import run_bass_kernel_spmd

F32 = mybir.dt.float32
BF16 = mybir.dt.bfloat16
AF = mybir.ActivationFunctionType

B, N, C, H, HD, W = 2, 2048, 1024, 16, 64, 8
SCALE = float(HD) ** -0.5
CORES = 8
TOK = 512            # token rows owned per core
HALO = TOK + 2 * W   # 528 k/v context tokens per core
NT = TOK // 128      # 4 row tiles of 128
WIN = 128 + 2 * W    # 144 k/v window per row tile

_CACHE = {}


def _build_nc(dbg=False):
    nc = bacc.Bacc(None, target_bir_lowering=False)
    xhT = nc.dram_tensor("xhT", [C, HALO], BF16, kind="ExternalInput")
    wqk = nc.dram_tensor("wqk", [C, 2 * C], BF16, kind="ExternalInput")
    wv = nc.dram_tensor("wv", [C, C], BF16, kind="ExternalInput")
    wp = nc.dram_tensor("wp", [C, C], BF16, kind="ExternalInput")
    bp = nc.dram_tensor("bp", [128, 8], F32, kind="ExternalInput")
    mA = nc.dram_tensor("mA", [128, TOK], BF16, kind="ExternalInput")
    mB = nc.dram_tensor("mB", [2 * W, TOK], BF16, kind="ExternalInput")
    outT = nc.dram_tensor("outT", [C, TOK], F32, kind="ExternalOutput")
    if dbg:
        qkTo = nc.dram_tensor("qkTo", [2 * C, HALO], BF16, kind="ExternalOutput")
        v1o = nc.dram_tensor("v1o", [HALO, H * (HD + 1)], BF16, kind="ExternalOutput")
        otno = nc.dram_tensor("otno", [C, TOK], BF16, kind="ExternalOutput")
        s_allo = nc.dram_tensor("s_allo", [64, 128], F32, kind="ExternalOutput")

    vsizes = [128, 128, 128, 128, 2 * W]

    with tile.TileContext(nc) as tc:
        with tc.tile_pool(name="persist", bufs=1) as pp:
            # ---- persistent SBUF arrays (bf16 weights fully resident) ----
            xh = [pp.tile([128, HALO], BF16, tag=f"xh{c}", name=f"xh{c}") for c in range(8)]
            for c in range(8):
                nc.sync.dma_start(out=xh[c][:], in_=xhT[128 * c:128 * (c + 1), :])
            wqk_sb = [pp.tile([128, 2 * C], BF16, tag=f"wqk{c}", name=f"wqk{c}") for c in range(8)]
            for c in range(8):
                nc.sync.dma_start(out=wqk_sb[c][:], in_=wqk[128 * c:128 * (c + 1), :])
            wv_sb = [pp.tile([128, C], BF16, tag=f"wv{c}", name=f"wv{c}") for c in range(8)]
            for c in range(8):
                nc.sync.dma_start(out=wv_sb[c][:], in_=wv[128 * c:128 * (c + 1), :])
            wp_sb = [pp.tile([128, C], BF16, tag=f"wp{c}", name=f"wp{c}") for c in range(8)]
            for c in range(8):
                nc.scalar.dma_start(out=wp_sb[c][:], in_=wp[128 * c:128 * (c + 1), :])
            mask_a = pp.tile([128, TOK], BF16, tag="mask_a", name="mask_a")
            mask_b = pp.tile([2 * W, TOK], BF16, tag="mask_b", name="mask_b")
            nc.scalar.dma_start(out=mask_a[:], in_=mA[:])
            nc.scalar.dma_start(out=mask_b[:], in_=mB[:])
            bias_sb = pp.tile([128, 8], F32, tag="bias", name="bias")
            nc.scalar.dma_start(out=bias_sb[:], in_=bp[:])

            qkT = [pp.tile([128, HALO], BF16, tag=f"qkT{m}", name=f"qkT{m}") for m in range(16)]
            v1 = [pp.tile([p, H, HD + 1], BF16, tag=f"v1_{t}", name=f"v1_{t}")
                  for t, p in enumerate(vsizes)]
            otn = [pp.tile([128, TOK], BF16, tag=f"otn{m}", name=f"otn{m}") for m in range(8)]
            s_all = pp.tile([64, 128], F32, tag="s_all", name="s_all")
            rec = pp.tile([64, 128], F32, tag="rec", name="rec")

            dp = tc.alloc_tile_pool(name="dram", bufs=1, space="DRAM")
            ds_all = dp.tile([H, TOK], F32, tag="ds_all", name="ds_all")
            drec = dp.tile([64, 128], F32, tag="drec", name="drec")

            # ---- P1: q,k projection -> qkT[m] = (Wqk[:, m-chunk]).T @ x^T ----
            with tc.tile_pool(name="psA", bufs=3, space="PSUM") as psA, \
                 tc.tile_pool(name="psB", bufs=3, space="PSUM") as psB:
                for m in range(16):
                    pa = psA.tile([128, 264], F32, tag="pa", name="pa")
                    pb = psB.tile([128, 264], F32, tag="pb", name="pb")
                    for c in range(8):
                        nc.tensor.matmul(pa[:], wqk_sb[c][:, 128 * m:128 * (m + 1)],
                                         xh[c][:, 0:264],
                                         start=(c == 0), stop=(c == 7))
                    for c in range(8):
                        nc.tensor.matmul(pb[:], wqk_sb[c][:, 128 * m:128 * (m + 1)],
                                         xh[c][:, 264:528],
                                         start=(c == 0), stop=(c == 7))
                    nc.vector.tensor_copy(qkT[m][:, 0:264], pa[:])
                    nc.vector.tensor_copy(qkT[m][:, 264:528], pb[:])

            # ---- P2: v projection (natural layout) + ones column ----
            with tc.tile_pool(name="psV", bufs=4, space="PSUM") as psV:
                for t in range(5):
                    p = vsizes[t]
                    pv0 = psV.tile([128, 512], F32, tag="pv0", name="pv0")
                    pv1 = psV.tile([128, 512], F32, tag="pv1", name="pv1")
                    for c in range(8):
                        nc.tensor.matmul(pv0[:p, :], xh[c][:, 128 * t:128 * t + p],
                                         wv_sb[c][:, 0:512],
                                         start=(c == 0), stop=(c == 7))
                    for c in range(8):
                        nc.tensor.matmul(pv1[:p, :], xh[c][:, 128 * t:128 * t + p],
                                         wv_sb[c][:, 512:1024],
                                         start=(c == 0), stop=(c == 7))
                    nc.vector.tensor_copy(
                        v1[t][:, 0:8, 0:HD],
                        pv0[:p, :].rearrange("p (h d) -> p h d", d=HD))
                    nc.vector.tensor_copy(
                        v1[t][:, 8:16, 0:HD],
                        pv1[:p, :].rearrange("p (h d) -> p h d", d=HD))
                    nc.vector.memset(v1[t][:, :, HD], 1.0)

            # ---- P3: banded attention, per-head [*, 512] strips ----
            with tc.tile_pool(name="psSA", bufs=2, space="PSUM") as psSA, \
                 tc.tile_pool(name="psSB", bufs=2, space="PSUM") as psSB, \
                 tc.tile_pool(name="psO", bufs=2, space="PSUM") as psO, \
                 tc.tile_pool(name="atpa", bufs=2) as atpa, \
                 tc.tile_pool(name="atpb", bufs=2) as atpb, \
                 tc.tile_pool(name="srp", bufs=2) as srp:
                for h in range(H):
                    fm = h // 2
                    off = (h % 2) * 64
                    stA = psSA.tile([128, TOK], F32, tag="stA", name="stA")
                    stB = psSB.tile([2 * W, TOK], F32, tag="stB", name="stB")
                    for t in range(NT):
                        q_ap = qkT[fm][off:off + 64, W + 128 * t:W + 128 * t + 128]
                        k1 = qkT[8 + fm][off:off + 64, 128 * t:128 * t + 128]
                        nc.tensor.matmul(stA[:, 128 * t:128 * (t + 1)], k1, q_ap,
                                         start=True, stop=True)
                    for t in range(NT):
                        q_ap = qkT[fm][off:off + 64, W + 128 * t:W + 128 * t + 128]
                        k2 = qkT[8 + fm][off:off + 64, 128 * t + 128:128 * t + WIN]
                        nc.tensor.matmul(stB[:, 128 * t:128 * (t + 1)], k2, q_ap,
                                         start=True, stop=True)
                    atA = atpa.tile([128, TOK], BF16, tag="atA", name="atA")
                    atB = atpb.tile([2 * W, TOK], BF16, tag="atB", name="atB")
                    nc.scalar.activation(atA[:], stA[:], AF.Exp)
                    nc.scalar.activation(atB[:], stB[:], AF.Exp)
                    nc.vector.tensor_mul(atA[:], atA[:], mask_a[:])
                    nc.vector.tensor_mul(atB[:], atB[:], mask_b[:])
                    otb = psO.tile([HD + 1, TOK], F32, tag="otb", name="otb")
                    for t in range(NT):
                        nc.tensor.matmul(otb[:, 128 * t:128 * (t + 1)],
                                         v1[t][:, h, :],
                                         atA[:, 128 * t:128 * (t + 1)],
                                         start=True, stop=False)
                        nc.tensor.matmul(otb[:, 128 * t:128 * (t + 1)],
                                         v1[t + 1][0:2 * W, h, :],
                                         atB[:, 128 * t:128 * (t + 1)],
                                         start=False, stop=True)
                    nc.vector.tensor_copy(otn[fm][off:off + 64, :], otb[0:HD, :])
                    srow = srp.tile([HD + 1, TOK], F32, tag="srow", name="srow")
                    nc.scalar.copy(srow[HD:HD + 1, :], otb[HD:HD + 1, :])
                    nc.sync.dma_start(out=ds_all[h:h + 1, :], in_=srow[HD:HD + 1, :])

            # ---- P4: normalization: otn *= broadcast(1/s_all) ----
            nc.sync.dma_start(
                out=s_all[:],
                in_=ds_all[:].rearrange("h (t c) -> (h t) c", c=128))
            nc.vector.reciprocal(rec[:], s_all[:])
            nc.sync.dma_start(out=drec[:], in_=rec[:])
            with tc.tile_pool(name="bcp", bufs=4) as bcp:
                dr_ap = drec[:]
                for fm in range(8):
                    bc = bcp.tile([128, TOK], F32, tag="bc", name="bc")
                    for j in range(2):
                        src = bass.AP(
                            tensor=dr_ap.tensor,
                            offset=dr_ap.offset + (8 * fm + 4 * j) * 128,
                            ap=[[0, 64], [128, NT], [1, 128]])
                        dst = bc[64 * j:64 * (j + 1), :].rearrange(
                            "b (t c) -> b t c", t=NT)
                        nc.sync.dma_start(out=dst, in_=src)
                    nc.vector.tensor_mul(otn[fm][:], otn[fm][:], bc[:])

            # ---- P5: output projection (transposed) + bias ----
            with tc.tile_pool(name="psf", bufs=2, space="PSUM") as psf, \
                 tc.tile_pool(name="outp", bufs=2) as outp:
                for m in range(8):
                    pf = psf.tile([128, 512], F32, tag="pf", name="pf")
                    for c in range(8):
                        nc.tensor.matmul(pf[:], wp_sb[c][:, 128 * m:128 * (m + 1)],
                                         otn[c][:],
                                         start=(c == 0), stop=(c == 7))
                    ob = outp.tile([128, 512], F32, tag="ob", name="ob")
                    nc.vector.tensor_scalar_add(ob[:], pf[:], bias_sb[:, m:m + 1])
                    nc.sync.dma_start(out=outT[128 * m:128 * (m + 1), :], in_=ob[:])

            if dbg:
                for m in range(16):
                    nc.sync.dma_start(out=qkTo[128 * m:128 * (m + 1), :], in_=qkT[m][:])
                for t in range(5):
                    p = vsizes[t]
                    nc.sync.dma_start(
                        out=v1o[128 * t:128 * t + p, :],
                        in_=v1[t][:].rearrange("p h d -> p (h d)"))
                for m in range(8):
                    nc.sync.dma_start(out=otno[128 * m:128 * (m + 1), :], in_=otn[m][:])
                nc.sync.dma_start(out=s_allo[:], in_=s_all[:])

    nc.finalize()
    return nc


def _get_nc(dbg=False):
    key = ("nc", dbg)
    if key not in _CACHE:
        _CACHE[key] = _build_nc(dbg)
    return _CACHE[key]


def _band_mask_np(n, w):
    i = np.arange(n)[:, None]
    j = np.arange(n)[None, :]
    lo = np.where(i <= w, 0, i - w)
    hi = np.where(n - i <= w, n - 1, i + w)
    return (j >= lo) & (j <= hi)


def _make_in_maps(x, Wqkv, Wproj, bproj):
    x = np.ascontiguousarray(np.asarray(x, dtype=np.float32))
    Wqkv = np.asarray(Wqkv, dtype=np.float32)
    Wproj = np.ascontiguousarray(np.asarray(Wproj, dtype=np.float32))
    bproj = np.asarray(bproj, dtype=np.float32)

    wqk_host = np.concatenate(
        [Wqkv[:, :C] * np.float32(SCALE), Wqkv[:, C:2 * C]], axis=1)
    wqk_host = np.ascontiguousarray(wqk_host).astype(ml_dtypes.bfloat16)
    wv_host = np.ascontiguousarray(Wqkv[:, 2 * C:]).astype(ml_dtypes.bfloat16)
    wp_host = Wproj.astype(ml_dtypes.bfloat16)
    bp_host = np.ascontiguousarray(bproj.reshape(8, 128).T)
    band = _band_mask_np(N, W)

    in_maps = []
    for core in range(CORES):
        b, qt = divmod(core, NT)
        g0 = qt * TOK
        xhrows = np.zeros((HALO, C), np.float32)
        s = max(0, g0 - W)
        e = min(N, g0 + TOK + W)
        xhrows[s - (g0 - W):e - (g0 - W)] = x[b, s:e]
        xhT_host = np.ascontiguousarray(xhrows.T).astype(ml_dtypes.bfloat16)

        mAh = np.zeros((128, TOK), np.float32)
        mBh = np.zeros((2 * W, TOK), np.float32)
        for t in range(NT):
            i = g0 + 128 * t + np.arange(128)[None, :]
            jw = (g0 - W) + 128 * t + np.arange(WIN)[:, None]
            valid = (jw >= 0) & (jw < N)
            mm = band[i, np.clip(jw, 0, N - 1)] & valid
            mAh[:, 128 * t:128 * (t + 1)] = mm[:128]
            mBh[:, 128 * t:128 * (t + 1)] = mm[128:]
        in_maps.append({
            "xhT": xhT_host, "wqk": wqk_host, "wv": wv_host,
            "wp": wp_host, "bp": bp_host,
            "mA": mAh.astype(ml_dtypes.bfloat16),
            "mB": mBh.astype(ml_dtypes.bfloat16),
        })
    return in_maps


def run_spmd(x, Wqkv, Wproj, bproj, dbg=False, **kw):
    """Run the SPMD kernel; returns (output, BassKernelResults)."""
    nc = _get_nc(dbg)
    in_maps = _make_in_maps(x, Wqkv, Wproj, bproj)
    res = run_bass_kernel_spmd(nc, in_maps, list(range(CORES)), **kw)
    outT = np.concatenate([res.results[i]["outT"] for i in range(CORES)], axis=1)
    out = np.ascontiguousarray(outT.T).reshape(B, N, C)
    return out, res


def kernel(x, Wqkv, Wproj, bproj):
    out, _ = run_spmd(x, Wqkv, Wproj, bproj)
    return out


# revision 23
# speedup vs baseline: 3.2618x; 1.1061x over previous
"""Banded (sparse) attention + projections on 8 Trainium2 NeuronCores.

Problem: nn_Attention_old_90211493085279
  x [2, 2048, 1024] -> qkv = x @ Wqkv, banded softmax(QK^T) V (half-width 8),
  out = attn @ Wproj + bproj.

Sharding choice: shard (batch x tokens) across the 8 cores -- each core owns a
contiguous block of 512 token rows (2 batches x 4 quarters). Because the
attention band is only 17 wide, each core needs just an 8-token halo of K/V
context, so there are NO collectives: every core computes QKV for its halo'd
token range (528 tokens), all 16 heads of banded attention for its own 512
rows, and the full output projection for its rows. The host concatenates the
per-core [1024, 512] transposed outputs.

Per-core pipeline (matmul operands bf16, accumulation f32), ordered to keep
the PE dense (HAM clock-gate stays warm):
  P2   v = x @ Wv in natural [token, head, dim+1] layout, 65th column = 1.0
       so the AV matmul also produces softmax denominators.
  loop over head-pairs fm: qk-chunk projections (feature-transposed qkT),
       then banded attention for heads 2fm, 2fm+1: transposed score strips
       st [window, 512 rows], at = exp(st) * bandmask, O^T strip [65, 512]
       = v1^T @ at, staged to SBUF; per-fm softmax normalization
       (sums -> DRAM -> reciprocal -> DRAM-broadcast -> fused mul into otn).
  P5   output projection consumes O^T directly; host re-transposes.
"""

import sys

sys.path.insert(0, "/opt/trn_rl_repo")

import ml_dtypes
import numpy as np

import concourse.bass as bass
import concourse.tile as tile
from concourse import bacc, mybir
from concourse.bass_utils import run_bass_kernel_spmd

F32 = mybir.dt.float32
BF16 = mybir.dt.bfloat16
AF = mybir.ActivationFunctionType

B, N, C, H, HD, W = 2, 2048, 1024, 16, 64, 8
SCALE = float(HD) ** -0.5
CORES = 8
TOK = 512            # token rows owned per core
HALO = TOK + 2 * W   # 528 k/v context tokens per core
NT = TOK // 128      # 4 row tiles of 128
WIN = 128 + 2 * W    # 144 k/v window per row tile

_CACHE = {}


def _build_nc(dbg=False):
    nc = bacc.Bacc(None, target_bir_lowering=False)
    xhT = nc.dram_tensor("xhT", [C, HALO], BF16, kind="ExternalInput")
    wqk = nc.dram_tensor("wqk", [C, 2 * C], BF16, kind="ExternalInput")
    wv = nc.dram_tensor("wv", [C, C], BF16, kind="ExternalInput")
    wp = nc.dram_tensor("wp", [C, C], BF16, kind="ExternalInput")
    bp = nc.dram_tensor("bp", [128, 8], F32, kind="ExternalInput")
    mA = nc.dram_tensor("mA", [128, TOK], BF16, kind="ExternalInput")
    mB = nc.dram_tensor("mB", [2 * W, TOK], BF16, kind="ExternalInput")
    outT = nc.dram_tensor("outT", [C, TOK], F32, kind="ExternalOutput")

    vsizes = [128, 128, 128, 128, 2 * W]

    with tile.TileContext(nc) as tc:
        with tc.tile_pool(name="persist", bufs=1) as pp:
            # ---- persistent SBUF arrays; small inputs first so they are not
            # stuck behind megabytes of weights in the DMA queues ----
            mask_a = pp.tile([128, TOK], BF16, tag="mask_a", name="mask_a")
            mask_b = pp.tile([2 * W, TOK], BF16, tag="mask_b", name="mask_b")
            bias_sb = pp.tile([128, 8], F32, tag="bias", name="bias")
            nc.sync.dma_start(out=mask_a[:], in_=mA[:])
            nc.sync.dma_start(out=mask_b[:], in_=mB[:])
            nc.sync.dma_start(out=bias_sb[:], in_=bp[:])
            xh = [pp.tile([128, HALO], BF16, tag=f"xh{c}", name=f"xh{c}") for c in range(8)]
            for c in range(8):
                nc.sync.dma_start(out=xh[c][:], in_=xhT[128 * c:128 * (c + 1), :])
            wv_sb = [pp.tile([128, C], BF16, tag=f"wv{c}", name=f"wv{c}") for c in range(8)]
            for c in range(8):
                nc.sync.dma_start(out=wv_sb[c][:], in_=wv[128 * c:128 * (c + 1), :])
            wqk_sb = [pp.tile([128, 2 * C], BF16, tag=f"wqk{c}", name=f"wqk{c}") for c in range(8)]
            for c in range(8):
                nc.sync.dma_start(out=wqk_sb[c][:], in_=wqk[128 * c:128 * (c + 1), :])
            wp_sb = [pp.tile([128, C], BF16, tag=f"wp{c}", name=f"wp{c}") for c in range(8)]
            for c in range(8):
                nc.scalar.dma_start(out=wp_sb[c][:], in_=wp[128 * c:128 * (c + 1), :])

            qkT = [pp.tile([128, HALO], BF16, tag=f"qkT{m}", name=f"qkT{m}") for m in range(16)]
            v1 = [pp.tile([p, H, HD + 1], BF16, tag=f"v1_{t}", name=f"v1_{t}")
                  for t, p in enumerate(vsizes)]
            otn = [pp.tile([128, TOK], BF16, tag=f"otn{m}", name=f"otn{m}") for m in range(8)]

            dp = tc.alloc_tile_pool(name="dram", bufs=1, space="DRAM")
            ds_all = dp.tile([H, TOK], F32, tag="ds_all", name="ds_all")
            drec = dp.tile([64, 128], F32, tag="drec", name="drec")

            # ---- P2: v projection (natural layout) + ones column ----
            with tc.tile_pool(name="psV", bufs=2, space="PSUM") as psV:
                for t in range(5):
                    p = vsizes[t]
                    pv0 = psV.tile([128, 512], F32, tag="pv0", name="pv0")
                    pv1 = psV.tile([128, 512], F32, tag="pv1", name="pv1")
                    for c in range(8):
                        nc.tensor.matmul(pv0[:p, :], xh[c][:, 128 * t:128 * t + p],
                                         wv_sb[c][:, 0:512],
                                         start=(c == 0), stop=(c == 7))
                    for c in range(8):
                        nc.tensor.matmul(pv1[:p, :], xh[c][:, 128 * t:128 * t + p],
                                         wv_sb[c][:, 512:1024],
                                         start=(c == 0), stop=(c == 7))
                    nc.vector.tensor_copy(
                        v1[t][:, 0:8, 0:HD],
                        pv0[:p, :].rearrange("p (h d) -> p h d", d=HD))
                    nc.vector.tensor_copy(
                        v1[t][:, 8:16, 0:HD],
                        pv1[:p, :].rearrange("p (h d) -> p h d", d=HD))
                    nc.vector.memset(v1[t][:, :, HD], 1.0)

            # ---- fused qk-projection + attention + normalization, per fm ----
            with tc.tile_pool(name="psA", bufs=2, space="PSUM") as psA, \
                 tc.tile_pool(name="psSA", bufs=2, space="PSUM") as psSA, \
                 tc.tile_pool(name="psSB", bufs=1, space="PSUM") as psSB, \
                 tc.tile_pool(name="psO", bufs=1, space="PSUM") as psO, \
                 tc.tile_pool(name="atpa", bufs=2) as atpa, \
                 tc.tile_pool(name="atpb", bufs=2) as atpb, \
                 tc.tile_pool(name="stgp", bufs=4) as stgp, \
                 tc.tile_pool(name="bcp", bufs=4) as bcp, \
                 tc.tile_pool(name="spp", bufs=2) as spp:
                stages = {}
                for fm in range(8):
                    # qk-chunk projections for this head pair: q chunk m=fm,
                    # k chunk m=8+fm
                    for m in (fm, 8 + fm):
                        pa = psA.tile([128, 264], F32, tag="pa", name="pa")
                        pb = psA.tile([128, 264], F32, tag="pb", name="pb")
                        for c in range(8):
                            nc.tensor.matmul(pa[:], wqk_sb[c][:, 128 * m:128 * (m + 1)],
                                             xh[c][:, 0:264],
                                             start=(c == 0), stop=(c == 7))
                        for c in range(8):
                            nc.tensor.matmul(pb[:], wqk_sb[c][:, 128 * m:128 * (m + 1)],
                                             xh[c][:, 264:528],
                                             start=(c == 0), stop=(c == 7))
                        eng = nc.vector if m < 8 else nc.scalar
                        if m < 8:
                            nc.vector.tensor_copy(qkT[m][:, 0:264], pa[:])
                            nc.vector.tensor_copy(qkT[m][:, 264:528], pb[:])
                        else:
                            nc.scalar.copy(qkT[m][:, 0:264], pa[:])
                            nc.scalar.copy(qkT[m][:, 264:528], pb[:])

                    for h in (2 * fm, 2 * fm + 1):
                        off = (h % 2) * 64
                        stA = psSA.tile([128, TOK], F32, tag="stA", name="stA")
                        stB = psSB.tile([2 * W, TOK], F32, tag="stB", name="stB")
                        for t in range(NT):
                            q_ap = qkT[fm][off:off + 64, W + 128 * t:W + 128 * t + 128]
                            k1 = qkT[8 + fm][off:off + 64, 128 * t:128 * t + 128]
                            nc.tensor.matmul(stA[:, 128 * t:128 * (t + 1)], k1, q_ap,
                                             start=True, stop=True)
                        for t in range(NT):
                            q_ap = qkT[fm][off:off + 64, W + 128 * t:W + 128 * t + 128]
                            k2 = qkT[8 + fm][off:off + 64, 128 * t + 128:128 * t + WIN]
                            nc.tensor.matmul(stB[:, 128 * t:128 * (t + 1)], k2, q_ap,
                                             start=True, stop=True)
                        atA = atpa.tile([128, TOK], BF16, tag="atA", name="atA")
                        atB = atpb.tile([2 * W, TOK], BF16, tag="atB", name="atB")
                        nc.scalar.activation(atA[:], stA[:], AF.Exp)
                        nc.scalar.activation(atB[:], stB[:], AF.Exp)
                        nc.vector.tensor_mul(atA[:], atA[:], mask_a[:])
                        nc.vector.tensor_mul(atB[:], atB[:], mask_b[:])
                        otb = psO.tile([HD + 1, TOK], F32, tag="otb", name="otb")
                        for t in range(NT):
                            nc.tensor.matmul(otb[:, 128 * t:128 * (t + 1)],
                                             v1[t][:, h, :],
                                             atA[:, 128 * t:128 * (t + 1)],
                                             start=True, stop=False)
                            nc.tensor.matmul(otb[:, 128 * t:128 * (t + 1)],
                                             v1[t + 1][0:2 * W, h, :],
                                             atB[:, 128 * t:128 * (t + 1)],
                                             start=False, stop=True)
                        stage = stgp.tile([HD + 1, TOK], F32, tag="stage",
                                          name="stage")
                        nc.scalar.copy(stage[:], otb[:])
                        nc.sync.dma_start(out=ds_all[h:h + 1, :],
                                          in_=stage[HD:HD + 1, :])
                        stages[h] = stage

                    # per-fm softmax normalization, fused with the otn store
                    s_part = spp.tile([8, 128], F32, tag="s_part", name="s_part")
                    nc.sync.dma_start(
                        out=s_part[:],
                        in_=ds_all[2 * fm:2 * fm + 2, :]
                        .rearrange("h (t c) -> (h t) c", c=128))
                    rec_p = spp.tile([8, 128], F32, tag="rec_p", name="rec_p")
                    nc.vector.reciprocal(rec_p[:], s_part[:])
                    nc.sync.dma_start(out=drec[8 * fm:8 * fm + 8, :], in_=rec_p[:])
                    dr_ap = drec[:]
                    for h in (2 * fm, 2 * fm + 1):
                        off = (h % 2) * 64
                        bc = bcp.tile([64, TOK], F32, tag="bc", name="bc")
                        src = bass.AP(
                            tensor=dr_ap.tensor,
                            offset=dr_ap.offset + 4 * h * 128,
                            ap=[[0, 64], [128, NT], [1, 128]])
                        nc.sync.dma_start(
                            out=bc[:].rearrange("b (t c) -> b t c", t=NT), in_=src)
                        nc.vector.tensor_mul(otn[fm][off:off + 64, :],
                                             stages[h][0:HD, :], bc[:])
                        del stages[h]

            # ---- P5: output projection (transposed) + bias ----
            with tc.tile_pool(name="psf", bufs=2, space="PSUM") as psf, \
                 tc.tile_pool(name="outp", bufs=2) as outp:
                for m in range(8):
                    pf = psf.tile([128, 512], F32, tag="pf", name="pf")
                    for c in range(8):
                        nc.tensor.matmul(pf[:], wp_sb[c][:, 128 * m:128 * (m + 1)],
                                         otn[c][:],
                                         start=(c == 0), stop=(c == 7))
                    ob = outp.tile([128, 512], F32, tag="ob", name="ob")
                    nc.vector.tensor_scalar_add(ob[:], pf[:], bias_sb[:, m:m + 1])
                    nc.sync.dma_start(out=outT[128 * m:128 * (m + 1), :], in_=ob[:])

    nc.finalize()
    return nc


def _get_nc(dbg=False):
    key = ("nc", dbg)
    if key not in _CACHE:
        _CACHE[key] = _build_nc(dbg)
    return _CACHE[key]


def _band_mask_np(n, w):
    i = np.arange(n)[:, None]
    j = np.arange(n)[None, :]
    lo = np.where(i <= w, 0, i - w)
    hi = np.where(n - i <= w, n - 1, i + w)
    return (j >= lo) & (j <= hi)


def _make_in_maps(x, Wqkv, Wproj, bproj):
    x = np.ascontiguousarray(np.asarray(x, dtype=np.float32))
    Wqkv = np.asarray(Wqkv, dtype=np.float32)
    Wproj = np.ascontiguousarray(np.asarray(Wproj, dtype=np.float32))
    bproj = np.asarray(bproj, dtype=np.float32)

    wqk_host = np.concatenate(
        [Wqkv[:, :C] * np.float32(SCALE), Wqkv[:, C:2 * C]], axis=1)
    wqk_host = np.ascontiguousarray(wqk_host).astype(ml_dtypes.bfloat16)
    wv_host = np.ascontiguousarray(Wqkv[:, 2 * C:]).astype(ml_dtypes.bfloat16)
    wp_host = Wproj.astype(ml_dtypes.bfloat16)
    bp_host = np.ascontiguousarray(bproj.reshape(8, 128).T)
    band = _band_mask_np(N, W)

    in_maps = []
    for core in range(CORES):
        b, qt = divmod(core, NT)
        g0 = qt * TOK
        xhrows = np.zeros((HALO, C), np.float32)
        s = max(0, g0 - W)
        e = min(N, g0 + TOK + W)
        xhrows[s - (g0 - W):e - (g0 - W)] = x[b, s:e]
        xhT_host = np.ascontiguousarray(xhrows.T).astype(ml_dtypes.bfloat16)

        mAh = np.zeros((128, TOK), np.float32)
        mBh = np.zeros((2 * W, TOK), np.float32)
        for t in range(NT):
            i = g0 + 128 * t + np.arange(128)[None, :]
            jw = (g0 - W) + 128 * t + np.arange(WIN)[:, None]
            valid = (jw >= 0) & (jw < N)
            mm = band[i, np.clip(jw, 0, N - 1)] & valid
            mAh[:, 128 * t:128 * (t + 1)] = mm[:128]
            mBh[:, 128 * t:128 * (t + 1)] = mm[128:]
        in_maps.append({
            "xhT": xhT_host, "wqk": wqk_host, "wv": wv_host,
            "wp": wp_host, "bp": bp_host,
            "mA": mAh.astype(ml_dtypes.bfloat16),
            "mB": mBh.astype(ml_dtypes.bfloat16),
        })
    return in_maps


def run_spmd(x, Wqkv, Wproj, bproj, dbg=False, **kw):
    """Run the SPMD kernel; returns (output, BassKernelResults)."""
    nc = _get_nc(dbg)
    in_maps = _make_in_maps(x, Wqkv, Wproj, bproj)
    res = run_bass_kernel_spmd(nc, in_maps, list(range(CORES)), **kw)
    outT = np.concatenate([res.results[i]["outT"] for i in range(CORES)], axis=1)
    out = np.ascontiguousarray(outT.T).reshape(B, N, C)
    return out, res


def kernel(x, Wqkv, Wproj, bproj):
    out, _ = run_spmd(x, Wqkv, Wproj, bproj)
    return out


# revision 25
# speedup vs baseline: 3.3205x; 1.0180x over previous
"""Banded (sparse) attention + projections on 8 Trainium2 NeuronCores.

Problem: nn_Attention_old_90211493085279
  x [2, 2048, 1024] -> qkv = x @ Wqkv, banded softmax(QK^T) V (half-width 8),
  out = attn @ Wproj + bproj.

Sharding choice: shard (batch x tokens) across the 8 cores -- each core owns a
contiguous block of 512 token rows (2 batches x 4 quarters). Because the
attention band is only 17 wide, each core needs just an 8-token halo of K/V
context, so there are NO collectives: every core computes QKV for its halo'd
token range (528 tokens), all 16 heads of banded attention for its own 512
rows, and the full output projection for its rows. The host concatenates the
per-core [1024, 512] transposed outputs.

Per-core pipeline (matmul operands bf16, accumulation f32), ordered to keep
the PE dense (HAM clock-gate stays warm):
  P2   v = x @ Wv in natural [token, head, dim+1] layout, 65th column = 1.0
       so the AV matmul also produces softmax denominators.
  loop over head-pairs fm: qk-chunk projections (feature-transposed qkT),
       then banded attention for heads 2fm, 2fm+1: transposed score strips
       st [window, 512 rows], at = exp(st) * bandmask, O^T strip [65, 512]
       = v1^T @ at, staged to SBUF; per-fm softmax normalization
       (sums -> DRAM -> reciprocal -> DRAM-broadcast -> fused mul into otn).
  P5   output projection consumes O^T directly; host re-transposes.
"""

import sys

sys.path.insert(0, "/opt/trn_rl_repo")

import ml_dtypes
import numpy as np

import concourse.bass as bass
import concourse.tile as tile
from concourse import bacc, mybir
from concourse.bass_utils import run_bass_kernel_spmd

F32 = mybir.dt.float32
BF16 = mybir.dt.bfloat16
AF = mybir.ActivationFunctionType

B, N, C, H, HD, W = 2, 2048, 1024, 16, 64, 8
SCALE = float(HD) ** -0.5
CORES = 8
TOK = 512            # token rows owned per core
HALO = TOK + 2 * W   # 528 k/v context tokens per core
NT = TOK // 128      # 4 row tiles of 128
WIN = 128 + 2 * W    # 144 k/v window per row tile

_CACHE = {}


def _build_nc(dbg=False):
    nc = bacc.Bacc(None, target_bir_lowering=False)
    xhT = nc.dram_tensor("xhT", [C, HALO], BF16, kind="ExternalInput")
    wqk = nc.dram_tensor("wqk", [C, 2 * C], BF16, kind="ExternalInput")
    wv = nc.dram_tensor("wv", [C, C], BF16, kind="ExternalInput")
    wp = nc.dram_tensor("wp", [C, C], BF16, kind="ExternalInput")
    bp = nc.dram_tensor("bp", [128, 8], F32, kind="ExternalInput")
    mA = nc.dram_tensor("mA", [128, TOK], BF16, kind="ExternalInput")
    mB = nc.dram_tensor("mB", [2 * W, TOK], BF16, kind="ExternalInput")
    outT = nc.dram_tensor("outT", [C, TOK], F32, kind="ExternalOutput")

    vsizes = [128, 128, 128, 128, 2 * W]

    with tile.TileContext(nc) as tc:
        with tc.tile_pool(name="persist", bufs=1) as pp:
            # ---- persistent SBUF arrays; small inputs first so they are not
            # stuck behind megabytes of weights in the DMA queues ----
            mask_a = pp.tile([128, TOK], BF16, tag="mask_a", name="mask_a")
            mask_b = pp.tile([2 * W, TOK], BF16, tag="mask_b", name="mask_b")
            bias_sb = pp.tile([128, 8], F32, tag="bias", name="bias")
            nc.sync.dma_start(out=mask_a[:], in_=mA[:])
            nc.sync.dma_start(out=mask_b[:], in_=mB[:])
            nc.sync.dma_start(out=bias_sb[:], in_=bp[:])
            xh = [pp.tile([128, HALO], BF16, tag=f"xh{c}", name=f"xh{c}") for c in range(8)]
            for c in range(8):
                nc.sync.dma_start(out=xh[c][:], in_=xhT[128 * c:128 * (c + 1), :])
            wv_sb = [pp.tile([128, C], BF16, tag=f"wv{c}", name=f"wv{c}") for c in range(8)]
            for c in range(8):
                nc.sync.dma_start(out=wv_sb[c][:, 0:512],
                                  in_=wv[128 * c:128 * (c + 1), 0:512])
            for c in range(8):
                nc.sync.dma_start(out=wv_sb[c][:, 512:1024],
                                  in_=wv[128 * c:128 * (c + 1), 512:1024])
            wqk_sb = [pp.tile([128, 2 * C], BF16, tag=f"wqk{c}", name=f"wqk{c}") for c in range(8)]
            for c in range(8):
                nc.sync.dma_start(out=wqk_sb[c][:], in_=wqk[128 * c:128 * (c + 1), :])
            wp_sb = [pp.tile([128, C], BF16, tag=f"wp{c}", name=f"wp{c}") for c in range(8)]
            for c in range(8):
                nc.scalar.dma_start(out=wp_sb[c][:], in_=wp[128 * c:128 * (c + 1), :])

            qkT = [pp.tile([128, HALO], BF16, tag=f"qkT{m}", name=f"qkT{m}") for m in range(16)]
            v1 = [pp.tile([p, H, HD + 1], BF16, tag=f"v1_{t}", name=f"v1_{t}")
                  for t, p in enumerate(vsizes)]
            otn = [pp.tile([128, TOK], BF16, tag=f"otn{m}", name=f"otn{m}") for m in range(8)]

            dp = tc.alloc_tile_pool(name="dram", bufs=1, space="DRAM")
            ds_all = dp.tile([H, TOK], F32, tag="ds_all", name="ds_all")
            drec = dp.tile([64, 128], F32, tag="drec", name="drec")

            # ---- P2: v projection (natural layout) + ones column ----
            with tc.tile_pool(name="psV", bufs=2, space="PSUM") as psV:
                for t in range(5):
                    p = vsizes[t]
                    pv0 = psV.tile([128, 512], F32, tag="pv0", name="pv0")
                    pv1 = psV.tile([128, 512], F32, tag="pv1", name="pv1")
                    for c in range(8):
                        nc.tensor.matmul(pv0[:p, :], xh[c][:, 128 * t:128 * t + p],
                                         wv_sb[c][:, 0:512],
                                         start=(c == 0), stop=(c == 7))
                    for c in range(8):
                        nc.tensor.matmul(pv1[:p, :], xh[c][:, 128 * t:128 * t + p],
                                         wv_sb[c][:, 512:1024],
                                         start=(c == 0), stop=(c == 7))
                    nc.vector.tensor_copy(
                        v1[t][:, 0:8, 0:HD],
                        pv0[:p, :].rearrange("p (h d) -> p h d", d=HD))
                    nc.vector.tensor_copy(
                        v1[t][:, 8:16, 0:HD],
                        pv1[:p, :].rearrange("p (h d) -> p h d", d=HD))
                    nc.vector.memset(v1[t][:, :, HD], 1.0)

            # ---- fused qk-projection + attention + normalization, per fm ----
            with tc.tile_pool(name="psA", bufs=1, space="PSUM") as psA, \
                 tc.tile_pool(name="psB", bufs=1, space="PSUM") as psB, \
                 tc.tile_pool(name="psSA", bufs=2, space="PSUM") as psSA, \
                 tc.tile_pool(name="psSB", bufs=1, space="PSUM") as psSB, \
                 tc.tile_pool(name="psO", bufs=1, space="PSUM") as psO, \
                 tc.tile_pool(name="atpa", bufs=2) as atpa, \
                 tc.tile_pool(name="atpb", bufs=2) as atpb, \
                 tc.tile_pool(name="stgp", bufs=4) as stgp, \
                 tc.tile_pool(name="bcp", bufs=4) as bcp, \
                 tc.tile_pool(name="spp", bufs=2) as spp:
                stages = {}
                for fm in range(8):
                    # qk-chunk projections for this head pair: q chunk m=fm
                    # (own tokens only), k chunk m=8+fm (full halo)
                    m = fm
                    pa = psA.tile([128, 512], F32, tag="pa", name="pa")
                    for c in range(8):
                        nc.tensor.matmul(pa[:], wqk_sb[c][:, 128 * m:128 * (m + 1)],
                                         xh[c][:, W:W + TOK],
                                         start=(c == 0), stop=(c == 7))
                    nc.vector.tensor_copy(qkT[m][:, W:W + TOK], pa[:])
                    m = 8 + fm
                    pk = psA.tile([128, 512], F32, tag="pk", name="pk")
                    pb = psB.tile([128, 2 * W], F32, tag="pb", name="pb")
                    for c in range(8):
                        nc.tensor.matmul(pk[:], wqk_sb[c][:, 128 * m:128 * (m + 1)],
                                         xh[c][:, 0:512],
                                         start=(c == 0), stop=(c == 7))
                        nc.tensor.matmul(pb[:], wqk_sb[c][:, 128 * m:128 * (m + 1)],
                                         xh[c][:, 512:528],
                                         start=(c == 0), stop=(c == 7))
                    nc.scalar.copy(qkT[m][:, 0:512], pk[:])
                    nc.scalar.copy(qkT[m][:, 512:528], pb[:])

                    for h in (2 * fm, 2 * fm + 1):
                        off = (h % 2) * 64
                        stA = psSA.tile([128, TOK], F32, tag="stA", name="stA")
                        stB = psSB.tile([2 * W, TOK], F32, tag="stB", name="stB")
                        for t in range(NT):
                            q_ap = qkT[fm][off:off + 64, W + 128 * t:W + 128 * t + 128]
                            k1 = qkT[8 + fm][off:off + 64, 128 * t:128 * t + 128]
                            nc.tensor.matmul(stA[:, 128 * t:128 * (t + 1)], k1, q_ap,
                                             start=True, stop=True)
                        for t in range(NT):
                            q_ap = qkT[fm][off:off + 64, W + 128 * t:W + 128 * t + 128]
                            k2 = qkT[8 + fm][off:off + 64, 128 * t + 128:128 * t + WIN]
                            nc.tensor.matmul(stB[:, 128 * t:128 * (t + 1)], k2, q_ap,
                                             start=True, stop=True)
                        atA = atpa.tile([128, TOK], BF16, tag="atA", name="atA")
                        atB = atpb.tile([2 * W, TOK], BF16, tag="atB", name="atB")
                        nc.scalar.activation(atA[:], stA[:], AF.Exp)
                        nc.scalar.activation(atB[:], stB[:], AF.Exp)
                        nc.vector.tensor_mul(atA[:], atA[:], mask_a[:])
                        nc.vector.tensor_mul(atB[:], atB[:], mask_b[:])
                        otb = psO.tile([HD + 1, TOK], F32, tag="otb", name="otb")
                        for t in range(NT):
                            nc.tensor.matmul(otb[:, 128 * t:128 * (t + 1)],
                                             v1[t][:, h, :],
                                             atA[:, 128 * t:128 * (t + 1)],
                                             start=True, stop=False)
                            nc.tensor.matmul(otb[:, 128 * t:128 * (t + 1)],
                                             v1[t + 1][0:2 * W, h, :],
                                             atB[:, 128 * t:128 * (t + 1)],
                                             start=False, stop=True)
                        stage = stgp.tile([HD + 1, TOK], F32, tag="stage",
                                          name="stage")
                        nc.scalar.copy(stage[:], otb[:])
                        nc.sync.dma_start(out=ds_all[h:h + 1, :],
                                          in_=stage[HD:HD + 1, :])
                        stages[h] = stage

                    # per-fm softmax normalization, fused with the otn store
                    s_part = spp.tile([8, 128], F32, tag="s_part", name="s_part")
                    nc.sync.dma_start(
                        out=s_part[:],
                        in_=ds_all[2 * fm:2 * fm + 2, :]
                        .rearrange("h (t c) -> (h t) c", c=128))
                    rec_p = spp.tile([8, 128], F32, tag="rec_p", name="rec_p")
                    nc.vector.reciprocal(rec_p[:], s_part[:])
                    nc.sync.dma_start(out=drec[8 * fm:8 * fm + 8, :], in_=rec_p[:])
                    dr_ap = drec[:]
                    for h in (2 * fm, 2 * fm + 1):
                        off = (h % 2) * 64
                        bc = bcp.tile([64, TOK], F32, tag="bc", name="bc")
                        src = bass.AP(
                            tensor=dr_ap.tensor,
                            offset=dr_ap.offset + 4 * h * 128,
                            ap=[[0, 64], [128, NT], [1, 128]])
                        nc.sync.dma_start(
                            out=bc[:].rearrange("b (t c) -> b t c", t=NT), in_=src)
                        nc.vector.tensor_mul(otn[fm][off:off + 64, :],
                                             stages[h][0:HD, :], bc[:])
                        del stages[h]

            # ---- P5: output projection (transposed) + bias ----
            with tc.tile_pool(name="psf", bufs=2, space="PSUM") as psf, \
                 tc.tile_pool(name="outp", bufs=2) as outp:
                for m in range(8):
                    pf = psf.tile([128, 512], F32, tag="pf", name="pf")
                    for c in range(8):
                        nc.tensor.matmul(pf[:], wp_sb[c][:, 128 * m:128 * (m + 1)],
                                         otn[c][:],
                                         start=(c == 0), stop=(c == 7))
                    ob = outp.tile([128, 512], F32, tag="ob", name="ob")
                    nc.vector.tensor_scalar_add(ob[:], pf[:], bias_sb[:, m:m + 1])
                    nc.sync.dma_start(out=outT[128 * m:128 * (m + 1), :], in_=ob[:])

    nc.finalize()
    return nc


def _get_nc(dbg=False):
    key = ("nc", dbg)
    if key not in _CACHE:
        _CACHE[key] = _build_nc(dbg)
    return _CACHE[key]


def _band_mask_np(n, w):
    i = np.arange(n)[:, None]
    j = np.arange(n)[None, :]
    lo = np.where(i <= w, 0, i - w)
    hi = np.where(n - i <= w, n - 1, i + w)
    return (j >= lo) & (j <= hi)


def _make_in_maps(x, Wqkv, Wproj, bproj):
    x = np.ascontiguousarray(np.asarray(x, dtype=np.float32))
    Wqkv = np.asarray(Wqkv, dtype=np.float32)
    Wproj = np.ascontiguousarray(np.asarray(Wproj, dtype=np.float32))
    bproj = np.asarray(bproj, dtype=np.float32)

    wqk_host = np.concatenate(
        [Wqkv[:, :C] * np.float32(SCALE), Wqkv[:, C:2 * C]], axis=1)
    wqk_host = np.ascontiguousarray(wqk_host).astype(ml_dtypes.bfloat16)
    wv_host = np.ascontiguousarray(Wqkv[:, 2 * C:]).astype(ml_dtypes.bfloat16)
    wp_host = Wproj.astype(ml_dtypes.bfloat16)
    bp_host = np.ascontiguousarray(bproj.reshape(8, 128).T)
    band = _band_mask_np(N, W)

    in_maps = []
    for core in range(CORES):
        b, qt = divmod(core, NT)
        g0 = qt * TOK
        xhrows = np.zeros((HALO, C), np.float32)
        s = max(0, g0 - W)
        e = min(N, g0 + TOK + W)
        xhrows[s - (g0 - W):e - (g0 - W)] = x[b, s:e]
        xhT_host = np.ascontiguousarray(xhrows.T).astype(ml_dtypes.bfloat16)

        mAh = np.zeros((128, TOK), np.float32)
        mBh = np.zeros((2 * W, TOK), np.float32)
        for t in range(NT):
            i = g0 + 128 * t + np.arange(128)[None, :]
            jw = (g0 - W) + 128 * t + np.arange(WIN)[:, None]
            valid = (jw >= 0) & (jw < N)
            mm = band[i, np.clip(jw, 0, N - 1)] & valid
            mAh[:, 128 * t:128 * (t + 1)] = mm[:128]
            mBh[:, 128 * t:128 * (t + 1)] = mm[128:]
        in_maps.append({
            "xhT": xhT_host, "wqk": wqk_host, "wv": wv_host,
            "wp": wp_host, "bp": bp_host,
            "mA": mAh.astype(ml_dtypes.bfloat16),
            "mB": mBh.astype(ml_dtypes.bfloat16),
        })
    return in_maps


def run_spmd(x, Wqkv, Wproj, bproj, dbg=False, **kw):
    """Run the SPMD kernel; returns (output, BassKernelResults)."""
    nc = _get_nc(dbg)
    in_maps = _make_in_maps(x, Wqkv, Wproj, bproj)
    res = run_bass_kernel_spmd(nc, in_maps, list(range(CORES)), **kw)
    outT = np.concatenate([res.results[i]["outT"] for i in range(CORES)], axis=1)
    out = np.ascontiguousarray(outT.T).reshape(B, N, C)
    return out, res


def kernel(x, Wqkv, Wproj, bproj):
    out, _ = run_spmd(x, Wqkv, Wproj, bproj)
    return out
